# revision 1
# baseline (speedup 1.0000x reference)
"""DGCNLayer (layer%2==0 branch) on 8 Trainium2 NeuronCores via Bass.

Math (per reference, with uv_vals == 1 per the problem spec and using
linearity to pull the dense GEMM past the segment-sum):
  User_n = leaky_relu(segsum_{rows}(vfea[cols]) @ W1 + b1, 0.1)
  Item_n = leaky_relu(segsum_{cols}(ufea[rows]) @ W2 + b2, 0.1)
  User_h = relu(concat([ufea, User_n]) @ Wu + bu)
  Item_h = relu(concat([vfea, Item_n]) @ Wi + bi)
  return stack([User_h, User_n, ufea, Item_h, Item_n, vfea])

Distribution: destination nodes are sharded 12500/core across the 8
cores (the sharding hint's destination-partitioned edge lists); the
gather tables (full vfea/ufea) are replicated into every core's HBM so
no collectives are needed. Per core and direction, edges are sorted by
(dst, src) and processed in 128-edge chunks: an indirect DMA gathers
the 128 source rows (one per partition), a one-hot selection matrix S
(built by a single tensor_scalar is_equal against an iota row) maps
edges to the 256 destinations of the current tile, and TensorE
accumulates psum[f, d] += msgs^T @ S over the tile's chunks. The dense
tail (W1/W2 matmul + LeakyReLU, union matmul + ReLU) consumes the
transposed aggregate directly from PSUM-staged SBUF tiles. Outputs are
written feature-major [128, 12544] and the host reassembles the stack.
"""
import sys
sys.path.insert(0, "/opt/trn_rl_repo")
import numpy as np

from concourse import bass, bacc, mybir
from concourse import bass_utils
from concourse.tile import TileContext

F32 = mybir.dt.float32
F32R = mybir.dt.float32r
BF16 = mybir.dt.bfloat16
I32 = mybir.dt.int32

NCORES = 8
N_NODES = 100000
SH = N_NODES // NCORES      # 12500 destinations per core
D = 128
TW = 256                    # dst-tile width (psum half-bank)
AGG_ROWS = 12544            # 12500 padded to x256
NT = AGG_ROWS // TW         # 49 dst tiles
GK = 4                      # 128-edge chunks gathered per indirect DMA
ALPHA = 0.1


def _prep_direction(dst_all: np.ndarray, src_all: np.ndarray):
    """Per-core edge lists sorted by (dst, src), tiled by 256 dsts, each
    tile's count padded to a cross-core-common multiple of 128.

    Returns (srcs, dstf, nchunks): srcs[c] int32 [128, EP/128] wrapped
    gather indices, dstf[c] float32 [128, EP/128] wrapped tile-relative
    dst (pad = -1), nchunks[t] = number of 128-chunks of tile t."""
    cores = []
    for c in range(NCORES):
        m = (dst_all >= c * SH) & (dst_all < (c + 1) * SH)
        d = dst_all[m] - c * SH
        s = src_all[m]
        o = np.lexsort((s, d))
        d, s = d[o], s[o]
        cnt = np.bincount(d // TW, minlength=NT)
        cores.append((d, s, cnt))

    nchunks = []
    for t in range(NT):
        mx = max(int(pc[2][t]) for pc in cores)
        nchunks.append(max(1, -(-mx // 128)))

    srcs, dstf = [], []
    for c in range(NCORES):
        d, s, cnt = cores[c]
        sp, dp = [], []
        off = 0
        for t in range(NT):
            n, p = int(cnt[t]), nchunks[t] * 128
            sk = np.zeros(p, np.int32)
            dk = np.full(p, -1.0, np.float32)
            sk[:n] = s[off:off + n]
            dk[:n] = (d[off:off + n] - t * TW).astype(np.float32)
            sp.append(sk)
            dp.append(dk)
            off += n
        sa = np.concatenate(sp)
        da = np.concatenate(dp)
        srcs.append(sa.reshape(-1, 128).T.copy())   # [128, EP/128]
        dstf.append(da.reshape(-1, 128).T.copy())
    return srcs, dstf, nchunks


def _build(nc: bass.Bass, nch_u, nch_i, epu: int, epi: int):
    vtab = nc.dram_tensor("vtab", [N_NODES, D], BF16, kind="ExternalInput")
    utab = nc.dram_tensor("utab", [N_NODES, D], BF16, kind="ExternalInput")
    ufeaT = nc.dram_tensor("ufeaT", [128, SH], F32, kind="ExternalInput")
    vfeaT = nc.dram_tensor("vfeaT", [128, SH], F32, kind="ExternalInput")
    gsu = nc.dram_tensor("gsu", [128, epu // 128], I32, kind="ExternalInput")
    gdu = nc.dram_tensor("gdu", [128, epu // 128], F32, kind="ExternalInput")
    gsi = nc.dram_tensor("gsi", [128, epi // 128], I32, kind="ExternalInput")
    gdi = nc.dram_tensor("gdi", [128, epi // 128], F32, kind="ExternalInput")
    iota = nc.dram_tensor("iota", [128, TW], F32, kind="ExternalInput")
    wn = {}
    for w in ("W1", "W2", "Wu_t", "Wu_b", "Wi_t", "Wi_b"):
        wn[w] = nc.dram_tensor(w, [128, 128], F32, kind="ExternalInput")
    for b in ("b1", "b2", "bu", "bi"):
        wn[b] = nc.dram_tensor(b, [128, 1], F32, kind="ExternalInput")

    unT = nc.dram_tensor("unT", [128, AGG_ROWS], F32, kind="ExternalOutput")
    uhT = nc.dram_tensor("uhT", [128, AGG_ROWS], F32, kind="ExternalOutput")
    inT = nc.dram_tensor("inT", [128, AGG_ROWS], F32, kind="ExternalOutput")
    ihT = nc.dram_tensor("ihT", [128, AGG_ROWS], F32, kind="ExternalOutput")

    with TileContext(nc) as tc:
        with (
            tc.tile_pool(name="wts", bufs=1) as wtsp,
            tc.tile_pool(name="idx", bufs=1) as idxp,
            tc.tile_pool(name="msg", bufs=16) as msgp,
            tc.tile_pool(name="sel", bufs=12) as selp,
            tc.tile_pool(name="cmp", bufs=4) as cmpp,
            tc.tile_pool(name="agg", bufs=3, space="PSUM") as aggp,
            tc.tile_pool(name="mmp", bufs=2, space="PSUM") as mmpp,
        ):
            w = {}
            for name in ("W1", "W2", "Wu_t", "Wu_b", "Wi_t", "Wi_b"):
                w[name] = wtsp.tile([128, 128], F32, tag=name, name=f"w_{name}")
                nc.sync.dma_start(w[name][:], wn[name][:])
            for name in ("b1", "b2", "bu", "bi"):
                w[name] = wtsp.tile([128, 1], F32, tag=name, name=f"w_{name}")
                nc.sync.dma_start(w[name][:], wn[name][:])
            t_iota = wtsp.tile([128, TW], F32, tag="iota")
            nc.sync.dma_start(t_iota[:], iota[:])

            t_gs, t_gd = {}, {}
            for key, gs, gd, ep in (("u", gsu, gdu, epu), ("i", gsi, gdi, epi)):
                t_gs[key] = idxp.tile([128, ep // 128], I32, tag=f"gs{key}", name=f"t_gs_{key}")
                t_gd[key] = idxp.tile([128, ep // 128], F32, tag=f"gd{key}", name=f"t_gd_{key}")
                nc.sync.dma_start(t_gs[key][:], gs[:])
                nc.sync.dma_start(t_gd[key][:], gd[:])

            def direction(key, table, nch, feaT, W1n, b1n, Wtn, Wbn, btn,
                          nT_out, hT_out):
                gs, gd = t_gs[key], t_gd[key]
                col = 0
                with nc.named_scope(f"dir_{key}"):
                    for t in range(NT):
                        n = nch[t]
                        psA = aggp.tile([128, TW], F32, tag="psA")
                        # per 128-edge chunk: indirect gather (one row per
                        # partition; HW only honors [128,1] offset APs),
                        # one-hot S, accumulate msgs^T @ S into psum
                        for c in range(n):
                            mt = msgp.tile([128, 128], BF16, tag="mt")
                            nc.gpsimd.indirect_dma_start(
                                out=mt[:], out_offset=None,
                                in_=table[:],
                                in_offset=bass.IndirectOffsetOnAxis(
                                    ap=gs[:, col + c:col + c + 1], axis=0),
                            )
                            st = selp.tile([128, TW], BF16, tag="st")
                            nc.vector.tensor_scalar(
                                st[:], t_iota[:],
                                gd[:, col + c:col + c + 1], None,
                                mybir.AluOpType.is_equal)
                            nc.tensor.matmul(
                                psA[:], mt[:], st[:],
                                start=(c == 0), stop=(c == n - 1))
                        col += n

                        # dense tail for this 256-dst tile
                        j0 = t * TW
                        aggT = cmpp.tile([128, TW], F32, tag="aggT")
                        nc.vector.tensor_copy(aggT[:], psA[:])
                        pn = mmpp.tile([128, TW], F32, tag="pn")
                        nc.tensor.matmul(pn[:], w[W1n][:], aggT[:],
                                         start=True, stop=True)
                        # leaky_relu(pn + b1) = max(y, alpha*y)
                        yt = cmpp.tile([128, TW], F32, tag="yt")
                        nc.vector.tensor_scalar_add(yt[:], pn[:], w[b1n][:])
                        zt = cmpp.tile([128, TW], F32, tag="zt")
                        nc.vector.tensor_scalar_mul(zt[:], yt[:], ALPHA)
                        nT = cmpp.tile([128, TW], F32, tag="nT")
                        nc.vector.tensor_tensor(nT[:], yt[:], zt[:],
                                                mybir.AluOpType.max)
                        nc.sync.dma_start(nT_out[:, j0:j0 + TW], nT[:])

                        ft = cmpp.tile([128, TW], F32, tag="ft")
                        fdt = min(TW, max(0, SH - j0))
                        if fdt < TW:
                            nc.vector.memset(ft[:], 0.0)
                        if fdt > 0:
                            nc.sync.dma_start(ft[:, :fdt], feaT[:, j0:j0 + fdt])
                        ph = mmpp.tile([128, TW], F32, tag="ph")
                        nc.tensor.matmul(ph[:], w[Wtn][:], ft[:],
                                         start=True, stop=False)
                        nc.tensor.matmul(ph[:], w[Wbn][:], nT[:],
                                         start=False, stop=True)
                        hT = cmpp.tile([128, TW], F32, tag="hT")
                        nc.scalar.activation(
                            hT[:], ph[:], mybir.ActivationFunctionType.Relu,
                            bias=w[btn][:], scale=1.0)
                        nc.sync.dma_start(hT_out[:, j0:j0 + TW], hT[:])

            direction("u", vtab, nch_u, ufeaT, "W1", "b1", "Wu_t", "Wu_b",
                      "bu", unT, uhT)
            direction("i", utab, nch_i, vfeaT, "W2", "b2", "Wi_t", "Wi_b",
                      "bi", inT, ihT)
    return nc


def _run(ufea, vfea, uv_rows, uv_cols, trace=False):
    su_l, du_l, nch_u = _prep_direction(uv_rows, uv_cols)
    si_l, di_l, nch_i = _prep_direction(uv_cols, uv_rows)
    epu = 128 * sum(nch_u)
    epi = 128 * sum(nch_i)

    nc = bacc.Bacc("TRN2", target_bir_lowering=False, debug=False,
                   dynamic_dma_scratch_size=2**16)
    _build(nc, nch_u, nch_i, epu, epi)
    nc.compile()

    import ml_dtypes
    common = {
        "vtab": vfea.astype(ml_dtypes.bfloat16),
        "utab": ufea.astype(ml_dtypes.bfloat16),
        "iota": np.tile(np.arange(TW, dtype=np.float32), (128, 1)),
    }
    in_maps = []
    for c in range(NCORES):
        m = dict(common)
        m["ufeaT"] = np.ascontiguousarray(ufea[c * SH:(c + 1) * SH].T)
        m["vfeaT"] = np.ascontiguousarray(vfea[c * SH:(c + 1) * SH].T)
        m["gsu"], m["gdu"] = su_l[c], du_l[c]
        m["gsi"], m["gdi"] = si_l[c], di_l[c]
        in_maps.append(m)
    return nc, in_maps


def kernel(ufea, vfea, uv_rows, uv_cols, uv_vals,
           W1, b1, W2, b2, Wu, bu, Wi, bi) -> np.ndarray:
    ufea = np.ascontiguousarray(np.asarray(ufea, np.float32))
    vfea = np.ascontiguousarray(np.asarray(vfea, np.float32))
    uv_rows = np.asarray(uv_rows, np.int32)
    uv_cols = np.asarray(uv_cols, np.int32)

    nc, in_maps = _run(ufea, vfea, uv_rows, uv_cols)
    Wu = np.asarray(Wu, np.float32)
    Wi = np.asarray(Wi, np.float32)
    for m in in_maps:
        m.update({
            "W1": np.asarray(W1, np.float32), "W2": np.asarray(W2, np.float32),
            "Wu_t": np.ascontiguousarray(Wu[:128]),
            "Wu_b": np.ascontiguousarray(Wu[128:]),
            "Wi_t": np.ascontiguousarray(Wi[:128]),
            "Wi_b": np.ascontiguousarray(Wi[128:]),
            "b1": np.asarray(b1, np.float32).reshape(128, 1),
            "b2": np.asarray(b2, np.float32).reshape(128, 1),
            "bu": np.asarray(bu, np.float32).reshape(128, 1),
            "bi": np.asarray(bi, np.float32).reshape(128, 1),
        })

    res = bass_utils.run_bass_kernel_spmd(nc, in_maps, list(range(NCORES)),
                                          trace=False)

    out = np.empty((6, N_NODES, D), np.float32)
    for c in range(NCORES):
        r = res.results[c]
        sl = slice(c * SH, (c + 1) * SH)
        out[0][sl] = r["uhT"][:, :SH].T
        out[1][sl] = r["unT"][:, :SH].T
        out[3][sl] = r["ihT"][:, :SH].T
        out[4][sl] = r["inT"][:, :SH].T
    out[2] = ufea
    out[5] = vfea
    return out

